# revision 77
# baseline (speedup 1.0000x reference)
import numpy as np

# nn_Attention: B=256, N=65, DIM=1024, HEADS=16, DH=64 across 8 cores (32 batches/core)
B, N, DIM, HEADS, DH = 256, 65, 1024, 16, 64
NCORES = 8
BPC = B // NCORES            # 32 batches per core
TOK = BPC * N                # 2080 tokens per core
BB = 4                       # batches per block
NBLK = BPC // BB             # 8 blocks
TB = BB * N                  # 260 tokens per block
BN_EPS = 1e-5


def _build(nc_mod, mybir, bass):
    f32 = mybir.dt.float32
    bf16 = mybir.dt.bfloat16
    Alu = mybir.AluOpType
    Act = mybir.ActivationFunctionType
    from concourse.tile import TileContext

    nc = nc_mod
    fp8 = mybir.dt.float8e4
    xt8 = nc.declare_dram_parameter("xt8", [DIM, TOK], fp8, isOutput=False)
    xt8l = nc.declare_dram_parameter("xt8l", [DIM, TOK], fp8, isOutput=False)
    wv8t = nc.declare_dram_parameter("wv8t", [DIM, DIM], fp8, isOutput=False)
    wv8lt = nc.declare_dram_parameter("wv8lt", [DIM, DIM], fp8, isOutput=False)
    wqk8t = nc.declare_dram_parameter("wqk8t", [DIM, 2 * DIM], fp8, isOutput=False)
    woutt = nc.declare_dram_parameter("woutt", [DIM, DIM], bf16, isOutput=False)
    wconvt = nc.declare_dram_parameter("wconvt", [9, 66, 65], bf16, isOutput=False)
    ident = nc.declare_dram_parameter("ident", [128, 128], bf16, isOutput=False)
    outT = nc.declare_dram_parameter("outT", [DIM, TOK], bf16, isOutput=True)

    from contextlib import ExitStack
    with TileContext(nc) as tc:
        with ExitStack() as es:
            P = lambda *a, **k: es.enter_context(tc.tile_pool(*a, **k))
            cp = P(name="consts", bufs=1)
            xtp = P(name="xtp", bufs=3)
            qtp = P(name="qtp", bufs=3)
            ktp = P(name="ktp", bufs=3)
            vtp = P(name="vtp", bufs=3)
            exp_p = P(name="exp", bufs=3)
            rcp = P(name="rcp", bufs=6)
            resp = P(name="resp", bufs=3)
            rtp = P(name="rtp", bufs=3)
            obp = P(name="obp", bufs=3)
            pqkv = P(name="pqkv", bufs=2, space="PSUM")
            ptp = P(name="ptp", bufs=1, space="PSUM")
            psp = P(name="psp", bufs=2, space="PSUM")
            pcf = P(name="pcf", bufs=2, space="PSUM")

            # ---- resident constants ----
            wv8_sb = cp.tile([128, 8 * DIM], fp8, tag="wv8")
            wv8 = wv8_sb[:].rearrange("p (a n) -> p a n", a=8)
            wv8l_sb = cp.tile([128, 8 * DIM], fp8, tag="wv8l")
            wv8l = wv8l_sb[:].rearrange("p (a n) -> p a n", a=8)
            wqk8_sb = cp.tile([128, 8 * 2 * DIM], fp8, tag="wqk8")
            wqk8 = wqk8_sb[:].rearrange("p (a n) -> p a n", a=8)
            for ki in range(8):
                eng = (nc.sync, nc.gpsimd, nc.scalar)[ki % 3]
                eng.dma_start(
                    out=wqk8[:, ki:ki + 1, 0:1024],
                    in_=wqk8t[:].rearrange(
                        "(a p) n -> p a n", p=128)[:, ki:ki + 1, 0:1024],
                )
            for ki in range(8):
                eng = (nc.scalar, nc.sync, nc.gpsimd)[ki % 3]
                eng.dma_start(
                    out=wqk8[:, ki:ki + 1, 1024:2048],
                    in_=wqk8t[:].rearrange(
                        "(a p) n -> p a n", p=128)[:, ki:ki + 1, 1024:2048],
                )
            for ki in range(8):
                eng2 = (nc.gpsimd, nc.scalar, nc.sync)[ki % 3]
                eng2.dma_start(
                    out=wv8[:, ki:ki + 1, :],
                    in_=wv8t[:].rearrange("(a p) n -> p a n", p=128)[:, ki:ki + 1, :],
                )
                eng2.dma_start(
                    out=wv8l[:, ki:ki + 1, :],
                    in_=wv8lt[:].rearrange("(a p) n -> p a n", p=128)[:, ki:ki + 1, :],
                )

            wout_sb = cp.tile([128, 8 * DIM], bf16, tag="wout")
            wout = wout_sb[:].rearrange("p (a n) -> p a n", a=8)

            wconv_sb = cp.tile([66, 9 * 65], bf16, tag="wconv")
            nc.sync.dma_start(
                out=wconv_sb[:].rearrange("i (t o) -> i t o", t=9),
                in_=wconvt[:].rearrange("t i o -> i t o"),
            )
            wconv = wconv_sb[:].rearrange("i (t o) -> i t o", t=9)

            id_sb = cp.tile([128, 128], bf16, tag="id")
            nc.sync.dma_start(out=id_sb[:], in_=ident[:])
            id32 = cp.tile([128, 128], f32, tag="id32")
            nc.scalar.activation(id32[:], id_sb[:], Act.Copy)

            # ---- persistent rotating v images, PACKED pitch-64 (18 rows of
            #      64: row 0 and 17 zero pads, rows 1..16 = heads), partition
            #      65 = ones (BN-shift row). Three dx-shifted variants so the
            #      transposed conv's stationary [66, 128] windows stay 2D
            #      contiguous; boundary columns of the shifted variants are
            #      pre-zeroed and never overwritten. ----
            vcs = []
            for vi in range(4):
                trio = []
                eng = nc.vector if vi < 2 else nc.gpsimd
                for sh in ("c", "m", "p"):
                    im = cp.tile([66, 18 * 64], bf16, tag=f"v{sh}{vi}",
                                 name=f"v{sh}{vi}")
                    imv = im[:].rearrange("p (h c) -> p h c", c=64)
                    eng.memset(im[64:66, :], 1.0)
                    eng.memset(im[0:65, 0:64], 0.0)
                    eng.memset(im[0:65, 17 * 64:18 * 64], 0.0)
                    if sh == "p":
                        eng.memset(imv[0:65, 1:17, 63:64], 0.0)
                    elif sh == "m":
                        eng.memset(imv[0:65, 1:17, 0:1], 0.0)
                    trio.append(im)
                vcs.append(trio)
            ones1 = cp.tile([65, 1], bf16, tag="ones1")
            nc.gpsimd.memset(ones1[:], 1.0)

            def _rt_chunk(p_res, bi, rt4v, imgs, c4):
                # transpose attention result chunk (f32) of one batch directly
                # into the conv's f32 psum accumulation group (transposed
                # conv: output [128 feat, 65 tok] per chunk), one copy out.
                img0, imgm, imgp = imgs
                pcv = pcf.tile([128, 512], f32, tag="pcf", name="pcv")
                for sub in range(2):
                    fc = 2 * c4 + sub
                    nc.tensor.matmul(
                        pcv[:, sub * 68:sub * 68 + N],
                        p_res[:, c4 * 256 + sub * 128:
                              c4 * 256 + sub * 128 + 128],
                        id32[0:65, 0:65],
                        is_transpose=True, start=True, stop=False,
                    )
                    for ti, t in enumerate((4, 0, 1, 2, 3, 5, 6, 7, 8)):
                        dy, dx = t // 3 - 1, t % 3 - 1
                        im = (imgm, img0, imgp)[dx + 1]
                        r0 = (1 + 2 * fc + dy) * 64
                        nc.tensor.matmul(
                            pcv[:, sub * 68:sub * 68 + N],
                            im[0:66, r0:r0 + 128],
                            wconv[:, t, :],
                            start=False, stop=(ti == 8),
                        )
                for sub in range(2):
                    fc = 2 * c4 + sub
                    dst = rt4v[:, fc, bi * N:(bi + 1) * N]
                    src = pcv[:, sub * 68:sub * 68 + N]
                    if sub == 0:
                        nc.vector.tensor_copy(dst, src)
                    else:
                        nc.scalar.activation(dst, src, Act.Copy)

            def proj_stream(rt4v, t0):
                # transposed out-projection for a whole block (4 batches)
                obT = obp.tile([128, 8 * TB], bf16, tag="ob", name="obT")
                obTv = obT[:].rearrange("p (a n) -> p a n", a=8)
                for c8 in range(8):
                    pfT = pcf.tile([128, 512], f32, tag="pcf", name="pfT")
                    for ki in range(8):
                        nc.tensor.matmul(
                            pfT[:, 0:TB],
                            wout[:, ki, c8 * 128:(c8 + 1) * 128],
                            rt4v[:, ki, :],
                            start=(ki == 0), stop=(ki == 7),
                        )
                    if c8 % 2 == 0:
                        nc.vector.tensor_copy(obTv[:, c8, :], pfT[:, 0:TB])
                    else:
                        nc.scalar.activation(obTv[:, c8, :], pfT[:, 0:TB], Act.Copy)
                    yield
                nc.sync.dma_start(
                    out=outT[:].rearrange(
                        "(a p) t -> p a t", p=128)[:, :, t0:t0 + TB],
                    in_=obTv,
                )

            DR = mybir.MatmulPerfMode.DoubleRow

            def _xdma(blk):
                t0 = blk * TB
                xt8_sb = xtp.tile([128, 8 * TB], fp8, tag="xt8", name="xt8_sb")
                xt8v = xt8_sb[:].rearrange("p (a n) -> p a n", a=8)
                xt8l_sb = xtp.tile([128, 8 * TB], fp8, tag="xt8l",
                                   name="xt8l_sb")
                xt8lv = xt8l_sb[:].rearrange("p (a n) -> p a n", a=8)
                nc.sync.dma_start(
                    out=xt8v,
                    in_=xt8[:].rearrange(
                        "(a p) n -> p a n", p=128)[:, :, t0:t0 + TB],
                )
                nc.gpsimd.dma_start(
                    out=xt8lv,
                    in_=xt8l[:].rearrange(
                        "(a p) n -> p a n", p=128)[:, :, t0:t0 + TB],
                )
                return xt8v, xt8lv

            def qkv_stream(xt8v, xt8lv, out):
                qt_sb = qtp.tile([64, 16 * TB], bf16, tag="qt", name="qt_sb")
                qtv = qt_sb[:].rearrange("p (h n) -> p h n", h=16)
                kt_sb = ktp.tile([64, 16 * TB], bf16, tag="kt", name="kt_sb")
                ktv = kt_sb[:].rearrange("p (h n) -> p h n", h=16)
                vt_sb = vtp.tile([128, 8 * TB], bf16, tag="vt", name="vt_sb")
                vtv = vt_sb[:].rearrange("p (a n) -> p a n", a=8)
                out["q"], out["k"], out["v"] = qtv, ktv, vtv
                for dst, coff in ((qtv, 0), (ktv, DIM), (vtv, 2 * DIM)):
                    for m in range(8):
                        pqk = pqkv.tile([128, 512], f32, tag="pq")
                        if dst is vtv:
                            for pi, (wop, xop) in enumerate(
                                    ((wv8, xt8v), (wv8, xt8lv), (wv8l, xt8v))):
                                for kt in range(4):
                                    nc.tensor.matmul(
                                        pqk[:, 0:TB],
                                        wop[:, 2 * kt:2 * kt + 2,
                                            m * 128:(m + 1) * 128],
                                        xop[:, 2 * kt:2 * kt + 2, :],
                                        start=(pi == 0 and kt == 0),
                                        stop=(pi == 2 and kt == 3),
                                        perf_mode=DR,
                                    )
                        else:
                            for kt in range(4):
                                nc.tensor.matmul(
                                    pqk[:, 0:TB],
                                    wqk8[:, 2 * kt:2 * kt + 2,
                                         coff + m * 128:coff + (m + 1) * 128],
                                    xt8v[:, 2 * kt:2 * kt + 2, :],
                                    start=(kt == 0), stop=(kt == 3),
                                    perf_mode=DR,
                                )
                        if dst is vtv:
                            nc.vector.tensor_scalar_mul(
                                dst[:, m, :], pqk[:, 0:TB], 1.0 / 256.0)
                        else:
                            nc.vector.tensor_copy(
                                dst[0:64, 2 * m, :], pqk[0:64, 0:TB])
                            nc.scalar.activation(
                                dst[0:64, 2 * m + 1, :], pqk[64:128, 0:TB],
                                Act.Copy)
                        yield

            nxt = {}
            with tc.high_priority():
                x0 = _xdma(0)
            for _ in qkv_stream(*x0, nxt):
                pass
            for blk in range(NBLK):
                t0 = blk * TB
                cur = nxt
                nxt = {}
                qtv, ktv, vtv = cur["q"], cur["k"], cur["v"]
                if blk == 1:
                    # wout is first needed by _proj(block 0) below; loading it
                    # here keeps the startup DMA window clear for qkv weights
                    nc.gpsimd.dma_start(
                        out=wout_sb[:].rearrange("p (a n) -> p a n", a=8),
                        in_=woutt[:].rearrange("(a p) n -> p a n", p=128),
                    )
                rt4 = rtp.tile([128, 8 * TB], bf16, tag="rt", name="rt4")
                rt4v = rt4[:].rearrange("p (k n) -> p k n", k=8)

                def batch_stream(bi):
                    toff = bi * N
                    img0, imgm, imgp = vcs[(blk * BB + bi) % 4]
                    im0v = img0[:].rearrange("p (h c) -> p h c", c=64)
                    immv = imgm[:].rearrange("p (h c) -> p h c", c=64)
                    impv = imgp[:].rearrange("p (h c) -> p h c", c=64)
                    # ---- v image: transpose v^T slices into [i, (h, x)] ----
                    for a2 in range(4):
                        pt = ptp.tile([128, 512], bf16, tag="pt", name="pt")
                        nc.tensor.transpose(
                            pt[0:65, 0:128],
                            vtv[:, 2 * a2, toff:toff + N], id_sb[:])
                        nc.tensor.transpose(
                            pt[0:65, 128:256],
                            vtv[:, 2 * a2 + 1, toff:toff + N], id_sb[:])
                        if a2 % 2 == 0:
                            nc.vector.tensor_copy(
                                img0[0:65, (1 + 4 * a2) * 64:(5 + 4 * a2) * 64],
                                pt[0:65, 0:256],
                            )
                        else:
                            nc.scalar.activation(
                                img0[0:65, (1 + 4 * a2) * 64:(5 + 4 * a2) * 64],
                                pt[0:65, 0:256],
                                Act.Copy,
                            )
                        if a2 == 1:
                            yield
                    # dx-shifted packed copies for the conv (boundary columns
                    # stay at their pre-zeroed values)
                    if blk == 0:
                        nc.vector.tensor_copy(
                            impv[0:65, 1:17, 0:63], im0v[0:65, 1:17, 1:64])
                        nc.scalar.activation(
                            immv[0:65, 1:17, 1:64], im0v[0:65, 1:17, 0:63],
                            Act.Copy)
                    else:
                        nc.gpsimd.tensor_copy(
                            impv[0:65, 1:17, 0:63], im0v[0:65, 1:17, 1:64])
                        nc.gpsimd.tensor_copy(
                            immv[0:65, 1:17, 1:64], im0v[0:65, 1:17, 0:63])
                    yield

                    res = resp.tile([65, DIM], f32, tag="res", name="res")
                    resv = res[:].rearrange("p (h c) -> p h c", c=64)
                    ex = exp_p.tile([65, 16 * 66], bf16, tag="ex", name="ex")

                    def _scores(g):
                        psc = psp.tile([65, 512], f32, tag="ps", name="psc")
                        for hi in range(4):
                            h = 4 * g + hi
                            nc.tensor.matmul(
                                psc[:, hi * 66:hi * 66 + N],
                                ktv[0:64, h, toff:toff + N],
                                qtv[0:64, h, toff:toff + N],
                                start=True, stop=True,
                            )
                        with tc.high_priority(offset=900):
                            nc.scalar.activation(
                                ex[:].rearrange("p (h c) -> p h c", c=66)[
                                    :, 4 * g:4 * g + 4, 0:65],
                                psc[:, 0:264].rearrange("p (h c) -> p h c", c=66)[
                                    :, :, 0:65],
                                Act.Exp, scale=float(DIM) ** -0.5 / 65536.0)

                    def _attnv(g):
                        po = psp.tile([65, 512], f32, tag="ps", name="po")
                        pov = po[:, 0:264].rearrange("p (h c) -> p h c", c=66)
                        for hi in range(4):
                            h = 4 * g + hi
                            nc.tensor.matmul(
                                po[:, hi * 66:hi * 66 + 64],
                                ex[:, h * 66:h * 66 + N],
                                img0[0:65, (1 + h) * 64:(2 + h) * 64],
                                start=True, stop=True,
                            )
                            nc.tensor.matmul(
                                po[:, hi * 66 + 65:hi * 66 + 66],
                                ex[:, h * 66:h * 66 + N],
                                ones1[:],
                                start=True, stop=True,
                            )
                        rc = rcp.tile([65, 4], f32, tag="rc", name="rc")
                        nc.vector.reciprocal(
                            rc[:].unsqueeze(2), pov[:, :, 65:66])
                        nc.vector.tensor_tensor(
                            resv[:, 4 * g:4 * g + 4, :],
                            pov[:, :, 0:64],
                            rc[:].unsqueeze(2).broadcast_to((65, 4, 64)),
                            Alu.mult,
                        )

                    for g in range(4):
                        _scores(g)
                        yield
                        _attnv(g)
                        yield
                    for c4 in range(4):
                        _rt_chunk(res, bi, rt4v, (img0, imgm, imgp), c4)
                        yield

                # modulo-scheduled emission: batch bi starts SKEW steps
                # after bi-1, so the in-order PE queue always holds
                # independent work; the next block's QKV and the previous
                # block's projection run as extra streams.
                SKEW = 5
                streams = [(batch_stream(b), b * SKEW) for b in range(BB)]
                if blk > 0:
                    streams.append((proj_stream(prev_rt4v, (blk - 1) * TB), 6))
                if blk + 1 < NBLK:
                    xn = _xdma(blk + 1)
                    streams.append((qkv_stream(*xn, nxt), 0))
                done = [False] * len(streams)
                step = 0
                while not all(done):
                    for i, (g, start) in enumerate(streams):
                        if done[i] or step < start:
                            continue
                        try:
                            next(g)
                        except StopIteration:
                            done[i] = True
                    step += 1
                prev_rt4v = rt4v
            for _ in proj_stream(prev_rt4v, (NBLK - 1) * TB):
                pass
    return nc


def kernel(x, w_qkv, b_qkv, w_out, b_out, conv_w, conv_b,
           bn_gamma, bn_beta, bn_mean, bn_var):
    import ml_dtypes
    import concourse.bass as bass
    import concourse.bacc as bacc
    import concourse.mybir as mybir
    from concourse.bass_utils import run_bass_kernel_spmd

    bf = ml_dtypes.bfloat16
    f8 = ml_dtypes.float8_e4m3
    x = np.asarray(x, np.float32)
    xt_f32 = np.ascontiguousarray(x.reshape(B * N, DIM).T)   # [1024, 16640]
    xt8_all = xt_f32.astype(f8)
    xt8l_all = (xt_f32 - xt8_all.astype(np.float32)).astype(f8)
    wqkv_f32 = np.asarray(w_qkv, np.float32).T               # [1024, 3072]
    wv_s = wqkv_f32[:, 2 * DIM:] * 256.0
    wv8t = np.ascontiguousarray(wv_s).astype(f8)
    wv8lt = np.ascontiguousarray(wv_s - wv8t.astype(np.float32)).astype(f8)
    wqk8t = np.ascontiguousarray(wqkv_f32[:, :2 * DIM] * 256.0).astype(f8)
    woutt_f32 = np.ascontiguousarray(np.asarray(w_out, np.float32).T)
    woutt = woutt_f32.astype(bf)

    s = np.asarray(bn_gamma, np.float32) / np.sqrt(np.asarray(bn_var, np.float32) + BN_EPS)
    t_aff = (np.asarray(conv_b, np.float32) - np.asarray(bn_mean, np.float32)) * s \
        + np.asarray(bn_beta, np.float32)
    wc = np.asarray(conv_w, np.float32).transpose(2, 3, 1, 0).reshape(9, N, N)
    wconvt = np.zeros((9, 66, N), np.float32)
    wconvt[:, :N, :] = wc * s[None, None, :]              # fold BN scale
    wconvt[4, N, :] = t_aff                               # BN shift via ones row
    wconvt = wconvt.astype(bf)
    identm = np.eye(128, dtype=np.float32).astype(bf)

    nc = bacc.Bacc()
    _build(nc, mybir, bass)
    nc.finalize()

    in_maps = []
    for c in range(NCORES):
        in_maps.append({
            "xt8": np.ascontiguousarray(xt8_all[:, c * TOK:(c + 1) * TOK]),
            "xt8l": np.ascontiguousarray(xt8l_all[:, c * TOK:(c + 1) * TOK]),
            "wv8t": wv8t, "wv8lt": wv8lt,
            "wqk8t": wqk8t, "woutt": woutt, "wconvt": wconvt,
            "ident": identm,
        })
    res = run_bass_kernel_spmd(nc, in_maps, list(range(NCORES)))
    global LAST_RESULTS
    LAST_RESULTS = res
    outs = [np.ascontiguousarray(res.results[c]["outT"].T).astype(np.float32)
            for c in range(NCORES)]
    full = np.concatenate(outs, axis=0).reshape(B, N, DIM)

    # exact host-side correction for v/out biases (batch-independent):
    # attn rows sum to 1 -> out1 += b_v; conv(v + b_v_img) = conv(v) + conv(b_v_img)
    b_v = np.asarray(b_qkv, np.float32)[2 * DIM:]
    bimg = b_v.reshape(HEADS, DH)
    pad = np.zeros((HEADS + 2, DH + 2), np.float32)
    pad[1:-1, 1:-1] = bimg
    wsum = np.asarray(conv_w, np.float32).sum(1)      # [65, 3, 3]
    dconv = np.zeros((N, HEADS, DH), np.float32)
    for ty in range(3):
        for tx in range(3):
            dconv += wsum[:, ty, tx][:, None, None] * \
                pad[ty:ty + HEADS, tx:tx + DH][None, :, :]
    dres = b_v[None, :] + (dconv * s[:, None, None]).reshape(N, DIM)
    dout = dres @ woutt_f32 + np.asarray(b_out, np.float32)[None, :]
    return full + dout[None, :, :]



# revision 80
# speedup vs baseline: 1.0037x; 1.0037x over previous
import numpy as np

# nn_Attention: B=256, N=65, DIM=1024, HEADS=16, DH=64 across 8 cores (32 batches/core)
B, N, DIM, HEADS, DH = 256, 65, 1024, 16, 64
NCORES = 8
BPC = B // NCORES            # 32 batches per core
TOK = BPC * N                # 2080 tokens per core
BB = 4                       # batches per block
NBLK = BPC // BB             # 8 blocks
TB = BB * N                  # 260 tokens per block
BN_EPS = 1e-5


def _build(nc_mod, mybir, bass):
    f32 = mybir.dt.float32
    bf16 = mybir.dt.bfloat16
    Alu = mybir.AluOpType
    Act = mybir.ActivationFunctionType
    from concourse.tile import TileContext

    nc = nc_mod
    fp8 = mybir.dt.float8e4
    xt8 = nc.declare_dram_parameter("xt8", [DIM, TOK], fp8, isOutput=False)
    xt8l = nc.declare_dram_parameter("xt8l", [DIM, TOK], fp8, isOutput=False)
    wv8t = nc.declare_dram_parameter("wv8t", [DIM, DIM], fp8, isOutput=False)
    wv8lt = nc.declare_dram_parameter("wv8lt", [DIM, DIM], fp8, isOutput=False)
    wqk8t = nc.declare_dram_parameter("wqk8t", [DIM, 2 * DIM], fp8, isOutput=False)
    woutt = nc.declare_dram_parameter("woutt", [DIM, DIM], bf16, isOutput=False)
    wconvt = nc.declare_dram_parameter("wconvt", [9, 66, 65], bf16, isOutput=False)
    ident = nc.declare_dram_parameter("ident", [128, 128], bf16, isOutput=False)
    outT = nc.declare_dram_parameter("outT", [DIM, TOK], bf16, isOutput=True)

    from contextlib import ExitStack
    with TileContext(nc) as tc:
        with ExitStack() as es:
            P = lambda *a, **k: es.enter_context(tc.tile_pool(*a, **k))
            cp = P(name="consts", bufs=1)
            xtp = P(name="xtp", bufs=3)
            qtp = P(name="qtp", bufs=3)
            ktp = P(name="ktp", bufs=3)
            vtp = P(name="vtp", bufs=3)
            exp_p = P(name="exp", bufs=3)
            rcp = P(name="rcp", bufs=6)
            resp = P(name="resp", bufs=3)
            rtp = P(name="rtp", bufs=3)
            obp = P(name="obp", bufs=3)
            pqkv = P(name="pqkv", bufs=2, space="PSUM")
            ptp = P(name="ptp", bufs=1, space="PSUM")
            psp = P(name="psp", bufs=2, space="PSUM")
            pcf = P(name="pcf", bufs=2, space="PSUM")

            # ---- resident constants ----
            wv8_sb = cp.tile([128, 8 * DIM], fp8, tag="wv8")
            wv8 = wv8_sb[:].rearrange("p (a n) -> p a n", a=8)
            wv8l_sb = cp.tile([128, 8 * DIM], fp8, tag="wv8l")
            wv8l = wv8l_sb[:].rearrange("p (a n) -> p a n", a=8)
            wqk8_sb = cp.tile([128, 8 * 2 * DIM], fp8, tag="wqk8")
            wqk8 = wqk8_sb[:].rearrange("p (a n) -> p a n", a=8)
            for ki in range(8):
                eng = (nc.sync, nc.gpsimd, nc.scalar)[ki % 3]
                eng.dma_start(
                    out=wqk8[:, ki:ki + 1, 0:1024],
                    in_=wqk8t[:].rearrange(
                        "(a p) n -> p a n", p=128)[:, ki:ki + 1, 0:1024],
                )
            for ki in range(8):
                eng = (nc.scalar, nc.sync, nc.gpsimd)[ki % 3]
                eng.dma_start(
                    out=wqk8[:, ki:ki + 1, 1024:2048],
                    in_=wqk8t[:].rearrange(
                        "(a p) n -> p a n", p=128)[:, ki:ki + 1, 1024:2048],
                )
            for ki in range(8):
                eng2 = (nc.gpsimd, nc.scalar, nc.sync)[ki % 3]
                eng2.dma_start(
                    out=wv8[:, ki:ki + 1, :],
                    in_=wv8t[:].rearrange("(a p) n -> p a n", p=128)[:, ki:ki + 1, :],
                )
                eng2.dma_start(
                    out=wv8l[:, ki:ki + 1, :],
                    in_=wv8lt[:].rearrange("(a p) n -> p a n", p=128)[:, ki:ki + 1, :],
                )

            wout_sb = cp.tile([128, 8 * DIM], bf16, tag="wout")
            wout = wout_sb[:].rearrange("p (a n) -> p a n", a=8)

            wconv_sb = cp.tile([66, 9 * 65], bf16, tag="wconv")
            nc.sync.dma_start(
                out=wconv_sb[:].rearrange("i (t o) -> i t o", t=9),
                in_=wconvt[:].rearrange("t i o -> i t o"),
            )
            wconv = wconv_sb[:].rearrange("i (t o) -> i t o", t=9)

            id_sb = cp.tile([128, 128], bf16, tag="id")
            nc.sync.dma_start(out=id_sb[:], in_=ident[:])
            id32 = cp.tile([128, 128], f32, tag="id32")
            nc.scalar.activation(id32[:], id_sb[:], Act.Copy)

            # ---- persistent rotating v images, PACKED pitch-64 (18 rows of
            #      64: row 0 and 17 zero pads, rows 1..16 = heads), partition
            #      65 = ones (BN-shift row). Three dx-shifted variants so the
            #      transposed conv's stationary [66, 128] windows stay 2D
            #      contiguous; boundary columns of the shifted variants are
            #      pre-zeroed and never overwritten. ----
            vcs = []
            for vi in range(4):
                trio = []
                eng = nc.vector if vi < 2 else nc.gpsimd
                for sh in ("c", "m", "p"):
                    im = cp.tile([66, 18 * 64], bf16, tag=f"v{sh}{vi}",
                                 name=f"v{sh}{vi}")
                    imv = im[:].rearrange("p (h c) -> p h c", c=64)
                    eng.memset(im[64:66, :], 1.0)
                    eng.memset(im[0:65, 0:64], 0.0)
                    eng.memset(im[0:65, 17 * 64:18 * 64], 0.0)
                    if sh == "p":
                        eng.memset(imv[0:65, 1:17, 63:64], 0.0)
                    elif sh == "m":
                        eng.memset(imv[0:65, 1:17, 0:1], 0.0)
                    trio.append(im)
                vcs.append(trio)
            ones1 = cp.tile([65, 1], bf16, tag="ones1")
            nc.gpsimd.memset(ones1[:], 1.0)

            def _rt_chunk(p_res, bi, rt4v, imgs, c4):
                # transpose attention result chunk (f32) of one batch directly
                # into the conv's f32 psum accumulation group (transposed
                # conv: output [128 feat, 65 tok] per chunk), one copy out.
                img0, imgm, imgp = imgs
                pcv = pcf.tile([128, 512], f32, tag="pcf", name="pcv")
                for sub in range(2):
                    fc = 2 * c4 + sub
                    nc.tensor.matmul(
                        pcv[:, sub * 68:sub * 68 + N],
                        p_res[:, c4 * 256 + sub * 128:
                              c4 * 256 + sub * 128 + 128],
                        id32[0:65, 0:65],
                        is_transpose=True, start=True, stop=False,
                    )
                    for ti, t in enumerate((4, 0, 1, 2, 3, 5, 6, 7, 8)):
                        dy, dx = t // 3 - 1, t % 3 - 1
                        im = (imgm, img0, imgp)[dx + 1]
                        r0 = (1 + 2 * fc + dy) * 64
                        nc.tensor.matmul(
                            pcv[:, sub * 68:sub * 68 + N],
                            im[0:66, r0:r0 + 128],
                            wconv[:, t, :],
                            start=False, stop=(ti == 8),
                        )
                for sub in range(2):
                    fc = 2 * c4 + sub
                    dst = rt4v[:, fc, bi * N:(bi + 1) * N]
                    src = pcv[:, sub * 68:sub * 68 + N]
                    if sub == 0:
                        nc.vector.tensor_copy(dst, src)
                    else:
                        nc.scalar.activation(dst, src, Act.Copy)

            def proj_stream(rt4v, t0):
                # transposed out-projection for a whole block (4 batches)
                obT = obp.tile([128, 8 * TB], bf16, tag="ob", name="obT")
                obTv = obT[:].rearrange("p (a n) -> p a n", a=8)
                for c8 in range(8):
                    pfT = pcf.tile([128, 512], f32, tag="pcf", name="pfT")
                    for ki in range(8):
                        nc.tensor.matmul(
                            pfT[:, 0:TB],
                            wout[:, ki, c8 * 128:(c8 + 1) * 128],
                            rt4v[:, ki, :],
                            start=(ki == 0), stop=(ki == 7),
                        )
                    if c8 % 2 == 0:
                        nc.vector.tensor_copy(obTv[:, c8, :], pfT[:, 0:TB])
                    else:
                        nc.scalar.activation(obTv[:, c8, :], pfT[:, 0:TB], Act.Copy)
                    yield
                nc.sync.dma_start(
                    out=outT[:].rearrange(
                        "(a p) t -> p a t", p=128)[:, :, t0:t0 + TB],
                    in_=obTv,
                )

            DR = mybir.MatmulPerfMode.DoubleRow

            def _xdma(blk):
                t0 = blk * TB
                xt8_sb = xtp.tile([128, 8 * TB], fp8, tag="xt8", name="xt8_sb")
                xt8v = xt8_sb[:].rearrange("p (a n) -> p a n", a=8)
                xt8l_sb = xtp.tile([128, 8 * TB], fp8, tag="xt8l",
                                   name="xt8l_sb")
                xt8lv = xt8l_sb[:].rearrange("p (a n) -> p a n", a=8)
                nc.sync.dma_start(
                    out=xt8v,
                    in_=xt8[:].rearrange(
                        "(a p) n -> p a n", p=128)[:, :, t0:t0 + TB],
                )
                nc.gpsimd.dma_start(
                    out=xt8lv,
                    in_=xt8l[:].rearrange(
                        "(a p) n -> p a n", p=128)[:, :, t0:t0 + TB],
                )
                return xt8v, xt8lv

            def qkv_stream(xt8v, xt8lv, out):
                qt_sb = qtp.tile([64, 16 * TB], bf16, tag="qt", name="qt_sb")
                qtv = qt_sb[:].rearrange("p (h n) -> p h n", h=16)
                kt_sb = ktp.tile([64, 16 * TB], bf16, tag="kt", name="kt_sb")
                ktv = kt_sb[:].rearrange("p (h n) -> p h n", h=16)
                vt_sb = vtp.tile([128, 8 * TB], bf16, tag="vt", name="vt_sb")
                vtv = vt_sb[:].rearrange("p (a n) -> p a n", a=8)
                out["q"], out["k"], out["v"] = qtv, ktv, vtv
                for dst, coff in ((qtv, 0), (ktv, DIM), (vtv, 2 * DIM)):
                    for m in range(8):
                        pqk = pqkv.tile([128, 512], f32, tag="pq")
                        if dst is vtv:
                            for pi, (wop, xop) in enumerate(
                                    ((wv8, xt8v), (wv8, xt8lv), (wv8l, xt8v))):
                                for kt in range(4):
                                    nc.tensor.matmul(
                                        pqk[:, 0:TB],
                                        wop[:, 2 * kt:2 * kt + 2,
                                            m * 128:(m + 1) * 128],
                                        xop[:, 2 * kt:2 * kt + 2, :],
                                        start=(pi == 0 and kt == 0),
                                        stop=(pi == 2 and kt == 3),
                                        perf_mode=DR,
                                    )
                        else:
                            for kt in range(4):
                                nc.tensor.matmul(
                                    pqk[:, 0:TB],
                                    wqk8[:, 2 * kt:2 * kt + 2,
                                         coff + m * 128:coff + (m + 1) * 128],
                                    xt8v[:, 2 * kt:2 * kt + 2, :],
                                    start=(kt == 0), stop=(kt == 3),
                                    perf_mode=DR,
                                )
                        if dst is vtv:
                            nc.vector.tensor_scalar_mul(
                                dst[:, m, :], pqk[:, 0:TB], 1.0 / 256.0)
                        else:
                            nc.vector.tensor_copy(
                                dst[0:64, 2 * m, :], pqk[0:64, 0:TB])
                            nc.scalar.activation(
                                dst[0:64, 2 * m + 1, :], pqk[64:128, 0:TB],
                                Act.Copy)
                        yield

            nxt = {}
            with tc.high_priority():
                x0 = _xdma(0)
            for _ in qkv_stream(*x0, nxt):
                pass
            for blk in range(NBLK):
                t0 = blk * TB
                cur = nxt
                nxt = {}
                qtv, ktv, vtv = cur["q"], cur["k"], cur["v"]
                if blk == 1:
                    # wout is first needed by _proj(block 0) below; loading it
                    # here keeps the startup DMA window clear for qkv weights
                    nc.gpsimd.dma_start(
                        out=wout_sb[:].rearrange("p (a n) -> p a n", a=8),
                        in_=woutt[:].rearrange("(a p) n -> p a n", p=128),
                    )
                rt4 = rtp.tile([128, 8 * TB], bf16, tag="rt", name="rt4")
                rt4v = rt4[:].rearrange("p (k n) -> p k n", k=8)

                def batch_stream(bi):
                    toff = bi * N
                    img0, imgm, imgp = vcs[(blk * BB + bi) % 4]
                    im0v = img0[:].rearrange("p (h c) -> p h c", c=64)
                    immv = imgm[:].rearrange("p (h c) -> p h c", c=64)
                    impv = imgp[:].rearrange("p (h c) -> p h c", c=64)
                    # block 0 only: scores/exp need just Q,K — run them while
                    # the V weights are still streaming in from HBM
                    early_scores = blk == 0
                    if early_scores:
                        res_e = resp.tile([65, DIM], f32, tag="res",
                                          name="res_e")
                        ex_e = exp_p.tile([65, 16 * 66], bf16, tag="ex",
                                          name="ex_e")
                        for g in range(4):
                            psc = psp.tile([65, 512], f32, tag="ps",
                                           name="psc")
                            for hi in range(4):
                                h = 4 * g + hi
                                nc.tensor.matmul(
                                    psc[:, hi * 66:hi * 66 + N],
                                    ktv[0:64, h, toff:toff + N],
                                    qtv[0:64, h, toff:toff + N],
                                    start=True, stop=True,
                                )
                            with tc.high_priority(offset=500):
                                nc.scalar.activation(
                                    ex_e[:].rearrange(
                                        "p (h c) -> p h c", c=66)[
                                        :, 4 * g:4 * g + 4, 0:65],
                                    psc[:, 0:264].rearrange(
                                        "p (h c) -> p h c", c=66)[:, :, 0:65],
                                    Act.Exp,
                                    scale=float(DIM) ** -0.5 / 65536.0)
                            yield
                    # ---- v image: transpose v^T slices into [i, (h, x)] ----
                    for a2 in range(4):
                        pt = ptp.tile([128, 512], bf16, tag="pt", name="pt")
                        nc.tensor.transpose(
                            pt[0:65, 0:128],
                            vtv[:, 2 * a2, toff:toff + N], id_sb[:])
                        nc.tensor.transpose(
                            pt[0:65, 128:256],
                            vtv[:, 2 * a2 + 1, toff:toff + N], id_sb[:])
                        if a2 % 2 == 0:
                            nc.vector.tensor_copy(
                                img0[0:65, (1 + 4 * a2) * 64:(5 + 4 * a2) * 64],
                                pt[0:65, 0:256],
                            )
                        else:
                            nc.scalar.activation(
                                img0[0:65, (1 + 4 * a2) * 64:(5 + 4 * a2) * 64],
                                pt[0:65, 0:256],
                                Act.Copy,
                            )
                        if a2 == 1:
                            yield
                    # dx-shifted packed copies for the conv (boundary columns
                    # stay at their pre-zeroed values)
                    if blk == 0:
                        nc.vector.tensor_copy(
                            impv[0:65, 1:17, 0:63], im0v[0:65, 1:17, 1:64])
                        nc.scalar.activation(
                            immv[0:65, 1:17, 1:64], im0v[0:65, 1:17, 0:63],
                            Act.Copy)
                    else:
                        nc.gpsimd.tensor_copy(
                            impv[0:65, 1:17, 0:63], im0v[0:65, 1:17, 1:64])
                        nc.gpsimd.tensor_copy(
                            immv[0:65, 1:17, 1:64], im0v[0:65, 1:17, 0:63])
                    yield

                    if early_scores:
                        res, ex = res_e, ex_e
                    else:
                        res = resp.tile([65, DIM], f32, tag="res", name="res")
                        ex = exp_p.tile([65, 16 * 66], bf16, tag="ex",
                                        name="ex")
                    resv = res[:].rearrange("p (h c) -> p h c", c=64)

                    def _scores(g):
                        psc = psp.tile([65, 512], f32, tag="ps", name="psc")
                        for hi in range(4):
                            h = 4 * g + hi
                            nc.tensor.matmul(
                                psc[:, hi * 66:hi * 66 + N],
                                ktv[0:64, h, toff:toff + N],
                                qtv[0:64, h, toff:toff + N],
                                start=True, stop=True,
                            )
                        with tc.high_priority(offset=900):
                            nc.scalar.activation(
                                ex[:].rearrange("p (h c) -> p h c", c=66)[
                                    :, 4 * g:4 * g + 4, 0:65],
                                psc[:, 0:264].rearrange("p (h c) -> p h c", c=66)[
                                    :, :, 0:65],
                                Act.Exp, scale=float(DIM) ** -0.5 / 65536.0)

                    def _attnv(g):
                        po = psp.tile([65, 512], f32, tag="ps", name="po")
                        pov = po[:, 0:264].rearrange("p (h c) -> p h c", c=66)
                        for hi in range(4):
                            h = 4 * g + hi
                            nc.tensor.matmul(
                                po[:, hi * 66:hi * 66 + 64],
                                ex[:, h * 66:h * 66 + N],
                                img0[0:65, (1 + h) * 64:(2 + h) * 64],
                                start=True, stop=True,
                            )
                            nc.tensor.matmul(
                                po[:, hi * 66 + 65:hi * 66 + 66],
                                ex[:, h * 66:h * 66 + N],
                                ones1[:],
                                start=True, stop=True,
                            )
                        rc = rcp.tile([65, 4], f32, tag="rc", name="rc")
                        nc.vector.reciprocal(
                            rc[:].unsqueeze(2), pov[:, :, 65:66])
                        nc.vector.tensor_tensor(
                            resv[:, 4 * g:4 * g + 4, :],
                            pov[:, :, 0:64],
                            rc[:].unsqueeze(2).broadcast_to((65, 4, 64)),
                            Alu.mult,
                        )

                    for g in range(4):
                        if not early_scores:
                            _scores(g)
                            yield
                        _attnv(g)
                        yield
                    for c4 in range(4):
                        _rt_chunk(res, bi, rt4v, (img0, imgm, imgp), c4)
                        yield

                # modulo-scheduled emission: batch bi starts SKEW steps
                # after bi-1, so the in-order PE queue always holds
                # independent work; the next block's QKV and the previous
                # block's projection run as extra streams.
                SKEW = 5
                streams = [(batch_stream(b), b * SKEW) for b in range(BB)]
                if blk > 0:
                    streams.append((proj_stream(prev_rt4v, (blk - 1) * TB), 6))
                if blk + 1 < NBLK:
                    xn = _xdma(blk + 1)
                    streams.append((qkv_stream(*xn, nxt), 0))
                done = [False] * len(streams)
                step = 0
                while not all(done):
                    for i, (g, start) in enumerate(streams):
                        if done[i] or step < start:
                            continue
                        try:
                            next(g)
                        except StopIteration:
                            done[i] = True
                    step += 1
                prev_rt4v = rt4v
            for _ in proj_stream(prev_rt4v, (NBLK - 1) * TB):
                pass
    return nc


def kernel(x, w_qkv, b_qkv, w_out, b_out, conv_w, conv_b,
           bn_gamma, bn_beta, bn_mean, bn_var):
    import ml_dtypes
    import concourse.bass as bass
    import concourse.bacc as bacc
    import concourse.mybir as mybir
    from concourse.bass_utils import run_bass_kernel_spmd

    bf = ml_dtypes.bfloat16
    f8 = ml_dtypes.float8_e4m3
    x = np.asarray(x, np.float32)
    xt_f32 = np.ascontiguousarray(x.reshape(B * N, DIM).T)   # [1024, 16640]
    xt8_all = xt_f32.astype(f8)
    xt8l_all = (xt_f32 - xt8_all.astype(np.float32)).astype(f8)
    wqkv_f32 = np.asarray(w_qkv, np.float32).T               # [1024, 3072]
    wv_s = wqkv_f32[:, 2 * DIM:] * 256.0
    wv8t = np.ascontiguousarray(wv_s).astype(f8)
    wv8lt = np.ascontiguousarray(wv_s - wv8t.astype(np.float32)).astype(f8)
    wqk8t = np.ascontiguousarray(wqkv_f32[:, :2 * DIM] * 256.0).astype(f8)
    woutt_f32 = np.ascontiguousarray(np.asarray(w_out, np.float32).T)
    woutt = woutt_f32.astype(bf)

    s = np.asarray(bn_gamma, np.float32) / np.sqrt(np.asarray(bn_var, np.float32) + BN_EPS)
    t_aff = (np.asarray(conv_b, np.float32) - np.asarray(bn_mean, np.float32)) * s \
        + np.asarray(bn_beta, np.float32)
    wc = np.asarray(conv_w, np.float32).transpose(2, 3, 1, 0).reshape(9, N, N)
    wconvt = np.zeros((9, 66, N), np.float32)
    wconvt[:, :N, :] = wc * s[None, None, :]              # fold BN scale
    wconvt[4, N, :] = t_aff                               # BN shift via ones row
    wconvt = wconvt.astype(bf)
    identm = np.eye(128, dtype=np.float32).astype(bf)

    nc = bacc.Bacc()
    _build(nc, mybir, bass)
    nc.finalize()

    in_maps = []
    for c in range(NCORES):
        in_maps.append({
            "xt8": np.ascontiguousarray(xt8_all[:, c * TOK:(c + 1) * TOK]),
            "xt8l": np.ascontiguousarray(xt8l_all[:, c * TOK:(c + 1) * TOK]),
            "wv8t": wv8t, "wv8lt": wv8lt,
            "wqk8t": wqk8t, "woutt": woutt, "wconvt": wconvt,
            "ident": identm,
        })
    res = run_bass_kernel_spmd(nc, in_maps, list(range(NCORES)))
    global LAST_RESULTS
    LAST_RESULTS = res
    outs = [np.ascontiguousarray(res.results[c]["outT"].T).astype(np.float32)
            for c in range(NCORES)]
    full = np.concatenate(outs, axis=0).reshape(B, N, DIM)

    # exact host-side correction for v/out biases (batch-independent):
    # attn rows sum to 1 -> out1 += b_v; conv(v + b_v_img) = conv(v) + conv(b_v_img)
    b_v = np.asarray(b_qkv, np.float32)[2 * DIM:]
    bimg = b_v.reshape(HEADS, DH)
    pad = np.zeros((HEADS + 2, DH + 2), np.float32)
    pad[1:-1, 1:-1] = bimg
    wsum = np.asarray(conv_w, np.float32).sum(1)      # [65, 3, 3]
    dconv = np.zeros((N, HEADS, DH), np.float32)
    for ty in range(3):
        for tx in range(3):
            dconv += wsum[:, ty, tx][:, None, None] * \
                pad[ty:ty + HEADS, tx:tx + DH][None, :, :]
    dres = b_v[None, :] + (dconv * s[:, None, None]).reshape(N, DIM)
    dout = dres @ woutt_f32 + np.asarray(b_out, np.float32)[None, :]
    return full + dout[None, :, :]

